# revision 1
# baseline (speedup 1.0000x reference)
"""DLinear Trainium2 kernel (nn_DLinear_45990509805636).

Math: with T=17 and KERNEL_SIZE=37 (PAD=18), every moving-average window
covers the whole sequence plus replicated edges, so

    trend[b,t,:] = (S + (18-t)*x0 + (t+2)*x16) / 37,   S = sum_t x[:,t,:]
    out = seasonal @ Ws[t] + trend @ Wt[t] + (bs+bt)[t]
        = x_t @ Ws[t] + trend_raw_t @ Wd[t] + bias[t],
    Wd = (Wt - Ws)/37 (host-folded), trend_raw_t = P + t*Q,
    P = S + 18*x0 + 2*x16, Q = x16 - x0.

Device per core (batch shard of 512 rows):
  - x.T resident in SBUF as [c%128, t, c//128, b]
  - S/P/Q/trend on DVE (bf16, 2x/4x modes)
  - per (b-tile, t): PSUM group = K=1 bias-broadcast matmul + 4 x@Ws
    matmuls (+ 4 trend@Wd matmuls, either same group or a second group
    joined by a DVE add when PHASE_SPLIT) -> ACT/DVE copy -> DMA out.

Sharding: data-parallel over batch, 8 cores x 512 rows; weights replicated.
"""

import os
import sys

sys.path.insert(0, "/opt/trn_rl_repo")

import numpy as np
import ml_dtypes

from concourse import bacc
import concourse.mybir as mybir
import concourse.tile as tile
from concourse.bass_utils import run_bass_kernel_spmd

dt = mybir.dt

B, T, C, D = 4096, 17, 512, 512
NCORES = 8
BC = B // NCORES          # 512 batch rows per core
KC = C // 128             # 4 contraction chunks
JB = BC // 128            # 4 output-row tiles per core

MODE = os.environ.get("DLINEAR_MODE", "bf16")


def build_bf16():
    # DRAM layouts are host-prepared so every DMA descriptor is a >=4KB
    # contiguous run: xt/wst/wdt are [t, c%128, c//128, {b,d}].
    idt = dt.bfloat16
    nc = bacc.Bacc(None, target_bir_lowering=False, name="dlinear_bf16")
    xt = nc.dram_tensor("xt", [T, 128, KC, BC], idt, kind="ExternalInput")
    wst = nc.dram_tensor("wst", [T, 128, KC, D], idt, kind="ExternalInput")
    wdt = nc.dram_tensor("wdt", [T, 128, KC, D], idt, kind="ExternalInput")
    bias = nc.dram_tensor("bias", [1, T * D], idt, kind="ExternalInput")
    out = nc.dram_tensor("out", [BC, T, D], dt.float16, kind="ExternalOutput")

    with tile.TileContext(nc) as tc:
        with (
            tc.tile_pool(name="xres", bufs=1) as xres,
            tc.tile_pool(name="consts", bufs=1) as consts,
            tc.tile_pool(name="stats", bufs=1) as stats,
            tc.tile_pool(name="wsbuf", bufs=4) as wsbuf,
            tc.tile_pool(name="wdbuf", bufs=3) as wdbuf,
            tc.tile_pool(name="tbuf", bufs=2) as tbuf,
            tc.tile_pool(name="abuf", bufs=44) as abuf,
            tc.tile_pool(name="obuf", bufs=4) as obuf,
            tc.tile_pool(name="psum_a", bufs=4, space="PSUM") as psum_a,
            tc.tile_pool(name="psum_b", bufs=4, space="PSUM") as psum_b,
        ):
            # SP dispatches dma_starts in-order at ~0.7us each and each
            # dma_start rides ONE ~20GB/s queue, so: few dispatches, sized
            # 128-512KB, issued in consumption order; out-stores dispatch
            # from the (otherwise idle) ACT sequencer.
            ones = consts.tile([1, 128], idt)
            nc.vector.memset(ones, 1.0)
            bsb = consts.tile([1, T * D], idt)
            nc.sync.dma_start(bsb, bias[:])

            xsb = xres.tile([128, T, KC, BC], idt)

            def emit_phase_a(t, wss, j):
                # bias + x@Ws; independent of S/trend, fills the prologue
                psa = psum_a.tile([128, D], dt.float32, tag="psa", name="psa")
                nc.tensor.matmul(psa, ones, bsb[:, t * D:(t + 1) * D],
                                 start=True, stop=False)
                for k in range(KC):
                    nc.tensor.matmul(
                        psa, xsb[:, t, k, j * 128:(j + 1) * 128], wss[:, k],
                        start=False, stop=(k == KC - 1),
                    )
                outa = abuf.tile([128, D], idt, tag="outa", name="outa")
                nc.scalar.copy(outa, psa)
                return outa

            PRE_T = 12
            ws_pre = {}
            for t in range(4):
                wss = wsbuf.tile([128, KC, D], idt, tag="ws", name="ws")
                nc.sync.dma_start(wss[:, 0:2], wst[t, :, 0:2])
                nc.sync.dma_start(wss[:, 2:4], wst[t, :, 2:4])
                ws_pre[t] = wss
            for t in range(T):
                nc.sync.dma_start(xsb[:, t, 0:2], xt[t, :, 0:2])
                nc.sync.dma_start(xsb[:, t, 2:4], xt[t, :, 2:4])
            # early wd so phase-B(0..3) isn't gated on late SP dispatch
            wd_pre = {}
            for t in range(3):
                wds = wdbuf.tile([128, KC, D], idt, tag="wd", name="wd")
                nc.sync.dma_start(wds[:, 0:2], wdt[t, :, 0:2])
                nc.sync.dma_start(wds[:, 2:4], wdt[t, :, 2:4])
                wd_pre[t] = wds
            for t in range(4, PRE_T):
                wss = wsbuf.tile([128, KC, D], idt, tag="ws", name="ws")
                nc.sync.dma_start(wss[:, 0:2], wst[t, :, 0:2])
                nc.sync.dma_start(wss[:, 2:4], wst[t, :, 2:4])
                ws_pre[t] = wss

            outa_pre = {}
            for t in range(PRE_T):
                for j in range(JB):
                    outa_pre[(t, j)] = emit_phase_a(t, ws_pre[t], j)

            # S/P/Q in bf16 on full [128, KC*BC] views (DVE 2x TT, 4x TS)
            S = stats.tile([128, KC, BC], idt)
            P = stats.tile([128, KC, BC], idt)
            Q = stats.tile([128, KC, BC], idt)
            nc.vector.tensor_tensor(S[:], xsb[:, 0], xsb[:, 1], mybir.AluOpType.add)
            for t in range(2, T):
                nc.vector.tensor_tensor(S[:], S[:], xsb[:, t], mybir.AluOpType.add)
            nc.vector.scalar_tensor_tensor(P[:], xsb[:, 0], 18.0, S[:],
                                           mybir.AluOpType.mult, mybir.AluOpType.add)
            nc.vector.scalar_tensor_tensor(P[:], xsb[:, 16], 2.0, P[:],
                                           mybir.AluOpType.mult, mybir.AluOpType.add)
            nc.vector.tensor_tensor(Q[:], xsb[:, 16], xsb[:, 0], mybir.AluOpType.subtract)

            osb_cur = {}
            for t in range(T):
                if t >= PRE_T:
                    wss = wsbuf.tile([128, KC, D], idt, tag="ws", name="ws")
                    nc.sync.dma_start(wss[:, 0:2], wst[t, :, 0:2])
                    nc.sync.dma_start(wss[:, 2:4], wst[t, :, 2:4])
                    ws_pre[t] = wss
                if t < 3:
                    wds = wd_pre.pop(t)
                else:
                    wds = wdbuf.tile([128, KC, D], idt, tag="wd", name="wd")
                    nc.sync.dma_start(wds[:, 0:2], wdt[t, :, 0:2])
                    nc.sync.dma_start(wds[:, 2:4], wdt[t, :, 2:4])

                trend = tbuf.tile([128, KC, BC], idt, tag="trend", name="trend")
                if t == 0:
                    nc.vector.tensor_copy(trend[:], P[:])
                else:
                    nc.vector.tensor_scalar_mul(trend[:], Q[:], float(t))
                    nc.vector.tensor_tensor(trend[:], trend[:], P[:], mybir.AluOpType.add)

                for j in range(JB):
                    if t < PRE_T:
                        outa = outa_pre.pop((t, j))
                    else:
                        outa = emit_phase_a(t, ws_pre[t], j)
                    psb = psum_b.tile([128, D], dt.float32, tag="psb", name="psb")
                    for k in range(KC):
                        nc.tensor.matmul(
                            psb, trend[:, k, j * 128:(j + 1) * 128], wds[:, k],
                            start=(k == 0), stop=(k == KC - 1),
                        )
                    # pair the stores: out[b, t-1:t+1, :] is contiguous, so
                    # buffer two tokens per [128, 2, D] tile and store once
                    # (dispatched from ACT to keep SP free for loads)
                    if t == T - 1:
                        osb = obuf.tile([128, 1, D], dt.float16, tag="osb1", name="osb1")
                        nc.vector.scalar_tensor_tensor(
                            osb[:, 0], psb, 1.0, outa,
                            mybir.AluOpType.mult, mybir.AluOpType.add,
                        )
                        nc.scalar.dma_start(
                            out[j * 128:(j + 1) * 128, t:t + 1, :], osb)
                    else:
                        if t % 2 == 0:
                            osb = obuf.tile([128, 2, D], dt.float16, tag="osb", name="osb")
                            osb_cur[j] = osb
                        else:
                            osb = osb_cur[j]
                        nc.vector.scalar_tensor_tensor(
                            osb[:, t % 2], psb, 1.0, outa,
                            mybir.AluOpType.mult, mybir.AluOpType.add,
                        )
                        if t % 2 == 1:
                            nc.scalar.dma_start(
                                out[j * 128:(j + 1) * 128, t - 1:t + 1, :], osb)
    nc.compile()
    return nc


def build_f32r():
    """x streamed twice in f32r; fp32-grade accuracy (~1.5e-4)."""
    idt = dt.float32r
    nc = bacc.Bacc(None, target_bir_lowering=False, name="dlinear_f32r")
    xt = nc.dram_tensor("xt", [T, C, BC], idt, kind="ExternalInput")
    wst = nc.dram_tensor("wst", [T, C, D], idt, kind="ExternalInput")
    wdt = nc.dram_tensor("wdt", [T, C, D], idt, kind="ExternalInput")
    bias = nc.dram_tensor("bias", [1, T * D], dt.bfloat16, kind="ExternalInput")
    out = nc.dram_tensor("out", [BC, T, D], dt.float32, kind="ExternalOutput")

    with tile.TileContext(nc) as tc:
        with (
            tc.tile_pool(name="consts", bufs=1) as consts,
            tc.tile_pool(name="stats", bufs=1) as stats,
            tc.tile_pool(name="spass", bufs=2) as spass,
            tc.tile_pool(name="xbuf", bufs=3) as xbuf,
            tc.tile_pool(name="wsbuf", bufs=3) as wsbuf,
            tc.tile_pool(name="wdbuf", bufs=3) as wdbuf,
            tc.tile_pool(name="tbuf", bufs=2) as tbuf,
            tc.tile_pool(name="obuf", bufs=8) as obuf,
            tc.tile_pool(name="psum", bufs=8, space="PSUM") as psum,
        ):
            ones = consts.tile([1, 128], dt.bfloat16)
            nc.vector.memset(ones, 1.0)
            bsb = consts.tile([1, T * D], dt.bfloat16)
            nc.sync.dma_start(bsb, bias[:])

            S = stats.tile([128, KC, BC], dt.float32)
            P = stats.tile([128, KC, BC], dt.float32)
            Q = stats.tile([128, KC, BC], dt.float32)
            for k in range(KC):
                xk = spass.tile([128, T, BC], idt, tag="xk")
                nc.sync.dma_start(
                    xk, xt[:, k * 128:(k + 1) * 128, :].rearrange("t p b -> p t b")
                )
                nc.vector.tensor_tensor(S[:, k], xk[:, 0], xk[:, 1], mybir.AluOpType.add)
                for t in range(2, T):
                    nc.vector.tensor_tensor(S[:, k], S[:, k], xk[:, t], mybir.AluOpType.add)
                nc.vector.scalar_tensor_tensor(
                    P[:, k], xk[:, 0], 18.0, S[:, k],
                    mybir.AluOpType.mult, mybir.AluOpType.add,
                )
                nc.vector.scalar_tensor_tensor(
                    P[:, k], xk[:, 16], 2.0, P[:, k],
                    mybir.AluOpType.mult, mybir.AluOpType.add,
                )
                nc.vector.scalar_tensor_tensor(
                    Q[:, k], xk[:, 0], -1.0, xk[:, 16],
                    mybir.AluOpType.mult, mybir.AluOpType.add,
                )

            for t in range(T):
                xts = xbuf.tile([128, KC, BC], idt, tag="xts")
                nc.sync.dma_start(xts, xt[t].rearrange("(k p) b -> p k b", p=128))
                wss = wsbuf.tile([128, KC, D], idt, tag="ws")
                nc.sync.dma_start(wss, wst[t].rearrange("(k p) d -> p k d", p=128))
                wds = wdbuf.tile([128, KC, D], idt, tag="wd")
                nc.sync.dma_start(wds, wdt[t].rearrange("(k p) d -> p k d", p=128))
                trend = tbuf.tile([128, KC, BC], idt, tag="trend")
                nc.vector.scalar_tensor_tensor(
                    trend[:], Q[:], float(t), P[:],
                    mybir.AluOpType.mult, mybir.AluOpType.add,
                )
                for j in range(JB):
                    ps = psum.tile([128, D], dt.float32, tag="ps")
                    nc.tensor.matmul(ps, ones, bsb[:, t * D:(t + 1) * D],
                                     start=True, stop=False)
                    for k in range(KC):
                        nc.tensor.matmul(
                            ps, xts[:, k, j * 128:(j + 1) * 128], wss[:, k],
                            start=False, stop=False,
                        )
                    for k in range(KC):
                        nc.tensor.matmul(
                            ps, trend[:, k, j * 128:(j + 1) * 128], wds[:, k],
                            start=False, stop=(k == KC - 1),
                        )
                    osb = obuf.tile([128, D], dt.float32, tag="osb")
                    nc.scalar.copy(osb, ps)
                    nc.sync.dma_start(out[j * 128:(j + 1) * 128, t, :], osb)
    nc.compile()
    return nc


_NC_CACHE = {}


def _get_nc(mode):
    if mode not in _NC_CACHE:
        _NC_CACHE[mode] = build_bf16() if mode == "bf16" else build_f32r()
    return _NC_CACHE[mode]


def kernel(x, W_seasonal, b_seasonal, W_trend, b_trend, _trace=False):
    mode = MODE
    npdt = ml_dtypes.bfloat16 if mode == "bf16" else np.float32
    nc = _get_nc(mode)

    def to_tpkd(w):  # [T, D, C] -> [T, 128, KC, D] (c-major on partitions)
        wt = w.transpose(0, 2, 1).reshape(T, KC, 128, D)
        return np.ascontiguousarray(wt.transpose(0, 2, 1, 3))

    if mode == "bf16":
        wst = to_tpkd(W_seasonal).astype(npdt)
        wdt = to_tpkd((W_trend - W_seasonal) / 37.0).astype(npdt)
    else:
        wst = np.ascontiguousarray(W_seasonal.transpose(0, 2, 1)).astype(npdt)
        wdt = np.ascontiguousarray(
            ((W_trend - W_seasonal) / 37.0).transpose(0, 2, 1)
        ).astype(npdt)
    bias = (b_seasonal + b_trend).reshape(1, T * D).astype(ml_dtypes.bfloat16)

    in_maps = []
    for i in range(NCORES):
        xs = x[i * BC:(i + 1) * BC]                    # [BC, T, C]
        if mode == "bf16":
            # [T, C, BC] -> [T, 128, KC, BC]
            xti = xs.transpose(1, 2, 0).reshape(T, KC, 128, BC)
            xti = np.ascontiguousarray(xti.transpose(0, 2, 1, 3)).astype(npdt)
        else:
            xti = np.ascontiguousarray(xs.transpose(1, 2, 0)).astype(npdt)
        in_maps.append({"xt": xti, "wst": wst, "wdt": wdt, "bias": bias})

    res = run_bass_kernel_spmd(
        nc, in_maps, core_ids=list(range(NCORES)), trace=_trace
    )
    outp = np.concatenate([r["out"] for r in res.results], axis=0)
    if outp.dtype != np.float32:
        outp = outp.astype(np.float32)
    if _trace:
        return outp, res
    return outp


if __name__ == "__main__":
    rng = np.random.default_rng(0)
    x = rng.standard_normal((B, T, C), dtype=np.float32)
    Ws = rng.uniform(-0.04, 0.04, (T, D, C)).astype(np.float32)
    Wt = rng.uniform(-0.04, 0.04, (T, D, C)).astype(np.float32)
    bs = rng.uniform(-0.04, 0.04, (T, D)).astype(np.float32)
    bt = rng.uniform(-0.04, 0.04, (T, D)).astype(np.float32)
    o = kernel(x, Ws, bs, Wt, bt)
    print("out shape:", o.shape, o.dtype)



# revision 2
# speedup vs baseline: 1.1379x; 1.1379x over previous
"""DLinear Trainium2 kernel (nn_DLinear_45990509805636).

Math: with T=17 and KERNEL_SIZE=37 (PAD=18), every moving-average window
covers the whole sequence plus replicated edges, so

    trend[b,t,:] = (S + (18-t)*x0 + (t+2)*x16) / 37,   S = sum_t x[:,t,:]
    out = x_t @ Ws[t] + trend_raw_t @ Wd[t] + bias[t],
    Wd = (Wt - Ws)/37 (host-folded), trend_raw_t = P + t*Q,
    P = S + 18*x0 + 2*x16, Q = x16 - x0.

Device mapping (per core, batch shard of 512 rows): weights are the
STATIONARY matmul operand and x/trend stream as the moving operand, so
PSUM holds out.T tiles [d_local=128, b=512] and the per-token bias
(which varies along d = the partition dim) is applied for free as the
ACT per-partition bias during PSUM eviction -- no bias matmuls.

  phase A per (t, dj): 4 matmuls  psa += Ws[t,k,dj].T @ x[t,k]
                       ACT: outa = psa + bias[t,dj]      (bf16)
  phase B per (t, dj): 4 matmuls  psb += Wd[t,k,dj].T @ trend[t,k]
                       DVE: osb = psb + outa             (f16)
  one 512KB store per t: out[t] = osb   ([T, KC, 128, BC] f16, host
  transposes back to [BC, T, D])

Sharding: data-parallel over batch, 8 cores x 512 rows; weights replicated.
"""

import sys

sys.path.insert(0, "/opt/trn_rl_repo")

import numpy as np
import ml_dtypes

from concourse import bacc
import concourse.mybir as mybir
import concourse.tile as tile
from concourse.bass_utils import run_bass_kernel_spmd

dt = mybir.dt

B, T, C, D = 4096, 17, 512, 512
NCORES = 8
BC = B // NCORES          # 512 batch rows per core
KC = C // 128             # 4 contraction chunks
DJ = D // 128             # 4 output-channel chunks (PSUM partition tiles)

PRE_T = 12                # phase-A tokens emitted before any phase-B work


def build():
    idt = dt.bfloat16
    nc = bacc.Bacc(None, target_bir_lowering=False, name="dlinear_v2")
    # p-major DRAM layouts: per-partition runs are large and contiguous
    xt = nc.dram_tensor("xt", [128, T, KC, BC], idt, kind="ExternalInput")
    wst = nc.dram_tensor("wst", [128, T, KC, D], idt, kind="ExternalInput")
    wdt = nc.dram_tensor("wdt", [128, T, KC, D], idt, kind="ExternalInput")
    biasc = nc.dram_tensor("biasc", [128, T * KC], dt.float32, kind="ExternalInput")
    out = nc.dram_tensor("out", [T, KC, 128, BC], dt.float16, kind="ExternalOutput")

    with tile.TileContext(nc) as tc:
        with (
            tc.tile_pool(name="xres", bufs=1) as xres,
            tc.tile_pool(name="consts", bufs=1) as consts,
            tc.tile_pool(name="stats", bufs=1) as stats,
            tc.tile_pool(name="wsbuf", bufs=2) as wsbuf,
            tc.tile_pool(name="wdbuf", bufs=2) as wdbuf,
            tc.tile_pool(name="tbuf", bufs=2) as tbuf,
            tc.tile_pool(name="abuf", bufs=4 * PRE_T + 4) as abuf,
            tc.tile_pool(name="obuf", bufs=2) as obuf,
            tc.tile_pool(name="psum_a", bufs=4, space="PSUM") as psum_a,
            tc.tile_pool(name="psum_b", bufs=4, space="PSUM") as psum_b,
        ):
            bsb = consts.tile([128, T * KC], dt.float32)
            nc.sync.dma_start(bsb, biasc[:])

            xsb = xres.tile([128, T, KC, BC], idt)

            # -- load schedule: ws/x interleaved t-wise (phase A consumes
            # both at ~290 GB/s), wd afterwards.  Few, large dispatches.
            ws_tiles = {}
            wd_tiles = {}

            def load_w(pool, dram, t0, t1, tag):
                w = pool.tile([128, t1 - t0, KC, D], idt, tag=tag, name=tag)
                nc.sync.dma_start(w, dram[:, t0:t1])
                return w, t0

            # first token of ws and x alone so the PE can start ASAP
            ws_tiles[0] = load_w(wsbuf, wst, 0, 2, "ws")
            nc.sync.dma_start(xsb[:, 0:1], xt[:, 0:1])
            nc.sync.dma_start(xsb[:, 1:3], xt[:, 1:3])
            ws_tiles[1] = load_w(wsbuf, wst, 2, 4, "ws")
            nc.sync.dma_start(xsb[:, 3:6], xt[:, 3:6])
            ws_tiles[2] = load_w(wsbuf, wst, 4, 6, "ws")
            nc.sync.dma_start(xsb[:, 6:9], xt[:, 6:9])
            ws_tiles[3] = load_w(wsbuf, wst, 6, 8, "ws")
            nc.sync.dma_start(xsb[:, 9:12], xt[:, 9:12])
            ws_tiles[4] = load_w(wsbuf, wst, 8, 10, "ws")
            nc.sync.dma_start(xsb[:, 12:15], xt[:, 12:15])
            ws_tiles[5] = load_w(wsbuf, wst, 10, 12, "ws")
            nc.sync.dma_start(xsb[:, 15:17], xt[:, 15:17])
            ws_order = [(12, 14), (14, 16), (16, 17)]
            wd_order = [(0, 2), (2, 4), (4, 6), (6, 8), (8, 10),
                        (10, 12), (12, 14), (14, 16), (16, 17)]

            def ws_for(t):
                return ws_tiles[t // 2]

            # -- stats on DVE (incremental as x lands)
            S = stats.tile([128, KC, BC], idt)
            P = stats.tile([128, KC, BC], idt)
            Q = stats.tile([128, KC, BC], idt)
            nc.vector.tensor_tensor(S[:], xsb[:, 0], xsb[:, 1], mybir.AluOpType.add)
            for t in range(2, T):
                nc.vector.tensor_tensor(S[:], S[:], xsb[:, t], mybir.AluOpType.add)
            nc.vector.scalar_tensor_tensor(P[:], xsb[:, 0], 18.0, S[:],
                                           mybir.AluOpType.mult, mybir.AluOpType.add)
            nc.vector.scalar_tensor_tensor(P[:], xsb[:, 16], 2.0, P[:],
                                           mybir.AluOpType.mult, mybir.AluOpType.add)
            nc.vector.tensor_tensor(Q[:], xsb[:, 16], xsb[:, 0], mybir.AluOpType.subtract)

            def emit_phase_a(t):
                wss, wt0 = ws_for(t)
                outs = []
                for dj in range(DJ):
                    psa = psum_a.tile([128, BC], dt.float32, tag="psa", name="psa")
                    for k in range(KC):
                        nc.tensor.matmul(
                            psa,
                            wss[:, t - wt0, k, dj * 128:(dj + 1) * 128],
                            xsb[:, t, k, :],
                            start=(k == 0), stop=(k == KC - 1),
                        )
                    outa = abuf.tile([128, BC], idt, tag="outa", name="outa")
                    nc.scalar.add(outa, psa, bsb[:, t * KC + dj:t * KC + dj + 1])
                    outs.append(outa)
                return outs

            outa_pre = {}
            for t in range(PRE_T):
                outa_pre[t] = emit_phase_a(t)

            # remaining weight loads (ws tail, then wd) -- dispatched after
            # the phase-A emission so SP issues them behind the early x/ws
            for t0, t1 in ws_order:
                ws_tiles[t0 // 2] = load_w(wsbuf, wst, t0, t1, "ws")
            for t0, t1 in wd_order:
                wd_tiles[t0 // 2] = load_w(wdbuf, wdt, t0, t1, "wd")

            for t in range(T):
                if t + PRE_T < T:
                    outa_pre[t + PRE_T] = emit_phase_a(t + PRE_T)
                wds, dt0 = wd_tiles[t // 2]

                trend = tbuf.tile([128, KC, BC], idt, tag="trend", name="trend")
                nc.vector.scalar_tensor_tensor(
                    trend[:], Q[:], float(t), P[:],
                    mybir.AluOpType.mult, mybir.AluOpType.add,
                )

                osb = obuf.tile([128, KC, BC], dt.float16, tag="osb", name="osb")
                outs = outa_pre.pop(t)
                for dj in range(DJ):
                    psb = psum_b.tile([128, BC], dt.float32, tag="psb", name="psb")
                    for k in range(KC):
                        nc.tensor.matmul(
                            psb,
                            wds[:, t - dt0, k, dj * 128:(dj + 1) * 128],
                            trend[:, k, :],
                            start=(k == 0), stop=(k == KC - 1),
                        )
                    nc.vector.scalar_tensor_tensor(
                        osb[:, dj], psb, 1.0, outs[dj],
                        mybir.AluOpType.mult, mybir.AluOpType.add,
                    )
                nc.scalar.dma_start(
                    out[t].rearrange("k p b -> p k b"), osb)
    nc.compile()
    return nc


_NC_CACHE = {}


def _get_nc():
    if "v2" not in _NC_CACHE:
        _NC_CACHE["v2"] = build()
    return _NC_CACHE["v2"]


def kernel(x, W_seasonal, b_seasonal, W_trend, b_trend, _trace=False):
    npdt = ml_dtypes.bfloat16
    nc = _get_nc()

    def to_pmajor_w(w):  # [T, D, C] -> [128, T, KC, D]  (c%128 on partitions)
        wt = w.transpose(2, 0, 1).reshape(KC, 128, T, D)
        return np.ascontiguousarray(wt.transpose(1, 2, 0, 3)).astype(npdt)

    wst = to_pmajor_w(W_seasonal)
    wdt = to_pmajor_w((W_trend - W_seasonal) / 37.0)
    btot = (b_seasonal + b_trend).astype(np.float32)          # [T, D]
    biasc = np.ascontiguousarray(
        btot.reshape(T, KC, 128).transpose(2, 0, 1).reshape(128, T * KC))

    in_maps = []
    for i in range(NCORES):
        xs = x[i * BC:(i + 1) * BC]                           # [BC, T, C]
        # [BC, T, C] -> [128, T, KC, BC]
        xti = xs.transpose(2, 1, 0).reshape(KC, 128, T, BC)
        xti = np.ascontiguousarray(xti.transpose(1, 2, 0, 3)).astype(npdt)
        in_maps.append({"xt": xti, "wst": wst, "wdt": wdt, "biasc": biasc})

    res = run_bass_kernel_spmd(
        nc, in_maps, core_ids=list(range(NCORES)), trace=_trace
    )
    # per-core out: [T, KC, 128, BC] f16 -> [BC, T, D]
    outp = np.concatenate(
        [r["out"].transpose(3, 0, 1, 2).reshape(BC, T, D) for r in res.results],
        axis=0,
    ).astype(np.float32)
    if _trace:
        return outp, res
    return outp


if __name__ == "__main__":
    rng = np.random.default_rng(0)
    x = rng.standard_normal((B, T, C), dtype=np.float32)
    Ws = rng.uniform(-0.04, 0.04, (T, D, C)).astype(np.float32)
    Wt = rng.uniform(-0.04, 0.04, (T, D, C)).astype(np.float32)
    bs = rng.uniform(-0.04, 0.04, (T, D)).astype(np.float32)
    bt = rng.uniform(-0.04, 0.04, (T, D)).astype(np.float32)
    o = kernel(x, Ws, bs, Wt, bt)

    # host reference check
    PAD = 18
    xp = np.concatenate([np.repeat(x[:, :1], PAD, 1), x,
                         np.repeat(x[:, -1:], PAD, 1)], axis=1)
    cs = np.cumsum(np.concatenate([np.zeros_like(xp[:, :1]), xp], 1), axis=1)
    trend = (cs[:, 37:] - cs[:, :-37]) / 37.0
    seasonal = x - trend
    ref = (np.einsum('btc,tdc->btd', seasonal, Ws) + bs[None]
           + np.einsum('btc,tdc->btd', trend, Wt) + bt[None])
    rel = np.linalg.norm(o - ref) / np.linalg.norm(ref)
    print("out shape:", o.shape, o.dtype, "rel err vs host ref:", rel)
